# revision 37
# baseline (speedup 1.0000x reference)
# CCAM channel-attention kernel for Trainium2 (Bass/Tile), 8-core SPMD.
#
# Math (per batch b):
#   q = x[b].reshape(C, N)                      # N = H*W = 4096
#   energy = q @ kbank                          # (C, 64), kbank = martx[0]
#   att = softmax(aphal * (rowmax(energy) - energy), axis=-1)
#   out = gamma * (att @ kbank.T) + x[b]
#
# Sharding: data-parallel over batch B=16 across 8 cores (2 batches/core);
# kbank, aphal, gamma are replicated.  aphal/gamma are baked into the
# program as immediates (cache keyed on their values).
#
# Per-core layout: the 2048 (b,c) rows are processed in 16 tiles of 128
# rows.  The contraction of matmul-1 runs over n, so q must be transposed
# on-chip: 32 PE transposes (fp32) per tile, cast to bf16 during the
# mandatory PSUM->SBUF copy (ScalarE).  Both matmuls run in bf16 (the
# attention output is a small residual correction to x, so bf16 error is
# negligible in the final fp32 output).  Softmax normalization and gamma
# are folded into the fused (psum * (gamma/s)) + x residual op on DVE.
#
# The kernel is HBM-bound (read 32 MB x + write out per core; compute is
# ~25% of the DMA time).  The output is therefore stored as bf16
# (out16=True): the store traffic halves (64->48 MB per core) for a
# relative error of ~3e-3 against the fp32 reference (tolerance 2e-2).
# The host upcasts to fp32 on return.  The key bank is passed only in
# transposed form (kbT_only: 16 KB contiguous DMA runs instead of 256 B
# rows, ~2x fewer descriptor cycles) and the chunked mm1 layout is
# rebuilt with PE transposes.  Loads/stores are split in half
# (split_in/split_out=2) so per-chunk dependencies overlap DMA with the
# transpose pipeline, and the final tile uses 8-way splits to drain the
# pipeline tail sooner.  TimelineSim puts this layout at ~156 us vs a
# ~148 us pure-DMA floor; chained HW runs measure ~165-174 us net of
# the ~130-250 us per-dispatch relay overhead.

import numpy as np
from contextlib import ExitStack

B, C = 16, 1024
HW = 4096          # H*W
KD = 64            # key bank dim
N_CORES = 8
P = 128            # partitions
ROWS = (B // N_CORES) * C   # 2048 rows per core
NT = ROWS // P              # 16 row tiles per core
NCH = HW // P               # 32 contraction chunks
NF = HW // 512              # 8 output free-dim chunks

_programs = {}

DEFAULT_CFG: dict = {
    "out16": True,
    "split_in": 2,
    "split_out": 2,
    # the final tile's load/store go in 8 finer chunks so the pipeline
    # drains sooner (the tail is the only stretch where DMA idles)
    "tail_n": 1,
    "tail_si": 8,
    "tail_so": 8,
    # load the key bank only in transposed form (16 KB contiguous runs per
    # partition instead of 256 B rows) and rebuild the chunked mm1 layout
    # with PE transposes; saves ~3 us of DMA descriptor overhead
    "kbT_only": True,
    # kbT_only moved the bank transposes off ps_a, so attT needs only one
    # PSUM buffer; the freed bank deepens the mm2/residual pipeline
    "psa_bufs": 1,
    "pso_bufs": 3,
}


def _build_program(aphal: float, gamma: float, cfg: dict | None = None):
    cfg = dict(DEFAULT_CFG) if cfg is None else cfg
    xs_bufs = cfg.get("xs_bufs", 5)
    qts_bufs = cfg.get("qts_bufs", 2)
    outs_bufs = cfg.get("outs_bufs", 2)
    pst_bufs = cfg.get("pst_bufs", 3)
    pse_bufs = cfg.get("pse_bufs", 1)
    psa_bufs = cfg.get("psa_bufs", 2)
    pso_bufs = cfg.get("pso_bufs", 2)
    split_in = cfg.get("split_in", 1)    # x load split per tile
    split_out = cfg.get("split_out", 1)  # out store split per tile
    dma_only = cfg.get("dma_only", False)  # timing-study mutant: no compute
    repeat = cfg.get("repeat", 1)        # timing-study: full passes per program
    out16 = cfg.get("out16", False)      # store output as bf16 (halves write BW)
    host_kbT = cfg.get("host_kbT", False)  # kbank^T passed as extra input
    kbT_only = cfg.get("kbT_only", False)  # ONLY kbT input; kb16 built on PE
    store_eng = cfg.get("store_eng", "sync")  # engine issuing output DMAs
    tail_n = cfg.get("tail_n", 0)        # last-N tiles get finer DMA splits
    tail_si = cfg.get("tail_si", 4)
    tail_so = cfg.get("tail_so", 8)
    tail_half = cfg.get("tail_half", 0)  # last-N tiles processed as 2x64 rows
    mm1_fp8 = cfg.get("mm1_fp8", False)  # energy matmul operands in fp8-e4m3
    prefetch = cfg.get("prefetch", 3)    # x loads emitted this many tiles ahead
    qt_chunk = cfg.get("qt_chunk", 4)    # transposes per PSUM group (4 or 8)
    res_chunk = cfg.get("res_chunk", 4)  # 128-col blocks per residual op (4 or 8)
    import concourse.mybir as mybir
    import concourse.tile as tile
    from concourse import bacc
    from concourse.masks import make_identity

    f32 = mybir.dt.float32
    bf16 = mybir.dt.bfloat16
    mm1_dt = mybir.dt.float8e4 if mm1_fp8 else bf16

    nc = bacc.Bacc(
        "TRN2",
        target_bir_lowering=False,
        debug=False,
        enable_asserts=False,
        num_devices=N_CORES,
    )
    x_d = nc.dram_tensor("x", (ROWS, HW), f32, kind="ExternalInput").ap()
    kb_d = (
        None
        if kbT_only
        else nc.dram_tensor("kb", (HW, KD), f32, kind="ExternalInput").ap()
    )
    kbT_d = (
        nc.dram_tensor("kbT", (KD, HW), f32, kind="ExternalInput").ap()
        if (host_kbT or kbT_only)
        else None
    )
    out_dt = bf16 if out16 else f32
    out_d = nc.dram_tensor("out", (ROWS, HW), out_dt, kind="ExternalOutput").ap()

    with tile.TileContext(nc) as tc, ExitStack() as ctx:
        const = ctx.enter_context(tc.tile_pool(name="const", bufs=1))
        xs = ctx.enter_context(tc.tile_pool(name="xs", bufs=xs_bufs))
        qts = ctx.enter_context(tc.tile_pool(name="qts", bufs=qts_bufs))
        outs = ctx.enter_context(tc.tile_pool(name="outs", bufs=outs_bufs))
        small = ctx.enter_context(tc.tile_pool(name="small", bufs=6))
        ps_t = ctx.enter_context(tc.tile_pool(name="ps_t", bufs=pst_bufs, space="PSUM"))
        ps_e = ctx.enter_context(tc.tile_pool(name="ps_e", bufs=pse_bufs, space="PSUM"))
        ps_a = ctx.enter_context(tc.tile_pool(name="ps_a", bufs=psa_bufs, space="PSUM"))
        ps_o = ctx.enter_context(tc.tile_pool(name="ps_o", bufs=pso_bufs, space="PSUM"))

        ident32 = const.tile([P, P], f32)
        make_identity(nc, ident32)
        ident16 = const.tile([P, P], bf16)
        make_identity(nc, ident16)

        # kb16[p, a, k] = kbank[a*128 + p, k] (mm1 rhs, chunked);
        # kbT16[k, n] = kbank^T (mm2 rhs).
        kb16 = const.tile([P, NCH, KD], mm1_dt)
        kbT16 = const.tile([KD, HW], bf16)
        if kbT_only:
            # kbank rows are only 256 B, so loading kb directly costs ~2x in
            # DMA descriptors; load the transposed bank (16 KB runs) and PE-
            # transpose it back into the chunked mm1 layout.
            kbTf = const.tile([KD, HW], f32)
            nc.sync.dma_start(out=kbTf, in_=kbT_d)
            nc.vector.tensor_copy(kbT16, kbTf)
            for grp in range(NCH // qt_chunk):
                psq = ps_t.tile([P, qt_chunk, P], f32)
                for j in range(qt_chunk):
                    a = qt_chunk * grp + j
                    nc.tensor.transpose(
                        psq[:, j, :KD],
                        kbTf[:, a * P:(a + 1) * P],
                        ident32[:KD, :KD],
                    )
                nc.scalar.copy(
                    kb16[:, qt_chunk * grp:qt_chunk * (grp + 1), :],
                    psq[:, :, :KD],
                )
        else:
            kb_sb = const.tile([P, NCH, KD], f32)
            nc.sync.dma_start(
                out=kb_sb, in_=kb_d.rearrange("(a p) k -> p a k", p=P)
            )
            nc.vector.tensor_copy(kb16, kb_sb)
            if host_kbT:
                kbTf = const.tile([KD, HW], f32)
                nc.sync.dma_start(out=kbTf, in_=kbT_d)
                nc.vector.tensor_copy(kbT16, kbTf)
            else:
                for a in range(NCH):
                    pst = ps_a.tile([KD, P], f32, tag="psa")
                    nc.tensor.transpose(pst, kb_sb[:, a, :], ident32)
                    nc.scalar.copy(kbT16[:, a * P:(a + 1) * P], pst)

        xts = {}
        NG = repeat * NT  # global tile count (repeat full passes)

        def load_x(g):
            t = g % NT
            si = tail_si if (tail_n and g >= NG - tail_n) else split_in
            xt = xs.tile([P, NCH, P], f32)
            x_src = x_d[t * P:(t + 1) * P, :].rearrange("p (a q) -> p a q", q=P)
            ci = NCH // si
            for s in range(si):
                nc.sync.dma_start(
                    out=xt[:, s * ci:(s + 1) * ci, :],
                    in_=x_src[:, s * ci:(s + 1) * ci, :],
                )
            xts[g] = xt

        for g in range(min(prefetch, NG)):
            load_x(g)

        for g in range(NG):
            t = g % NT
            # --- load x tile (128 rows x 4096) ---
            if g + prefetch < NG:
                load_x(g + prefetch)
            elif g not in xts:
                load_x(g)
            xt = xts.pop(g)

            if dma_only:
                o_dst = out_d[t * P:(t + 1) * P, :].rearrange(
                    "p (a q) -> p a q", q=P
                )
                if out16:
                    zt = outs.tile([P, NCH, P], bf16)
                    nc.vector.tensor_copy(zt, xt)
                    nc.sync.dma_start(out=o_dst, in_=zt)
                else:
                    nc.sync.dma_start(out=o_dst, in_=xt)
                continue

            # --- transpose q: 32 PE transposes, qt_chunk per PSUM group,
            # cast to the mm1 dtype during the PSUM->SBUF copy ---
            qT16 = qts.tile([P, NCH, P], mm1_dt)
            for grp in range(NCH // qt_chunk):
                psq = ps_t.tile([P, qt_chunk, P], f32)
                for j in range(qt_chunk):
                    a = qt_chunk * grp + j
                    nc.tensor.transpose(psq[:, j, :], xt[:, a, :], ident32)
                nc.scalar.copy(
                    qT16[:, qt_chunk * grp:qt_chunk * (grp + 1), :], psq
                )

            # --- energy = q @ kbank : accumulate over 32 chunks ---
            pse = ps_e.tile([P, KD], f32)
            for a in range(NCH):
                nc.tensor.matmul(
                    pse,
                    lhsT=qT16[:, a, :],
                    rhs=kb16[:, a, :],
                    start=(a == 0),
                    stop=(a == NCH - 1),
                )

            # --- inverted softmax: exp(aphal*(max - e)), unnormalized ---
            mx = small.tile([P, 1], f32)
            nc.vector.reduce_max(mx, pse, axis=mybir.AxisListType.X)
            mxs = small.tile([P, 1], f32)
            nc.vector.tensor_scalar_mul(mxs, mx, float(aphal))
            att16 = small.tile([P, KD], bf16)
            ssum = small.tile([P, 1], f32)
            nc.scalar.activation(
                att16,
                pse,
                mybir.ActivationFunctionType.Exp,
                bias=mxs,
                scale=-float(aphal),
                accum_out=ssum,
            )
            rinv = small.tile([P, 1], f32)
            nc.vector.reciprocal(rinv, ssum)
            rg = small.tile([P, 1], f32)
            nc.vector.tensor_scalar_mul(rg, rinv, float(gamma))

            # --- att^T (PE transpose, bf16) ---
            psa = ps_a.tile([KD, P], bf16, tag="psa")
            nc.tensor.transpose(psa, att16, ident16)
            attT = small.tile([KD, P], bf16)
            nc.scalar.copy(attT, psa)

            # --- out = (att @ kbank^T) * (gamma/s) + x ;  DMA out ---
            # On the final tile(s) the post-softmax chain is the pipeline
            # drain, so it optionally runs as two 64-row halves: the first
            # half's stores start while the second half still computes.
            ot = outs.tile([P, NCH, P], out_dt)
            mm_per_res = res_chunk // 4  # matmuls (N=512) per residual op
            is_tail = tail_half and g >= NG - tail_half
            halves = ((0, P // 2), (P // 2, P)) if is_tail else ((0, P),)
            store = nc.scalar if store_eng == "scalar" else nc.sync
            so = tail_so if (tail_n and g >= NG - tail_n) else split_out
            co = NCH // so
            for lo, hi in halves:
                for r in range(NCH // res_chunk):
                    pso = ps_o.tile([P, res_chunk, P], f32)
                    for m in range(mm_per_res):
                        nf = r * mm_per_res + m
                        nc.tensor.matmul(
                            pso[lo:hi, 4 * m:4 * (m + 1), :],
                            lhsT=attT[:, lo:hi],
                            rhs=kbT16[:, nf * 512:(nf + 1) * 512],
                            start=True,
                            stop=True,
                        )
                    nc.vector.scalar_tensor_tensor(
                        out=ot[lo:hi, res_chunk * r:res_chunk * (r + 1), :],
                        in0=pso[lo:hi],
                        scalar=rg[lo:hi],
                        in1=xt[lo:hi, res_chunk * r:res_chunk * (r + 1), :],
                        op0=mybir.AluOpType.mult,
                        op1=mybir.AluOpType.add,
                    )
                o_dst = out_d[t * P + lo:t * P + hi, :].rearrange(
                    "p (a q) -> p a q", q=P
                )
                for s in range(so):
                    store.dma_start(
                        out=o_dst[:, s * co:(s + 1) * co, :],
                        in_=ot[lo:hi, s * co:(s + 1) * co, :],
                    )

    nc.compile()
    return nc


def _get_program(aphal: float, gamma: float):
    key = (aphal, gamma)
    if key not in _programs:
        _programs[key] = _build_program(aphal, gamma)
    return _programs[key]


def run(x, martx, aphal, gamma, trace=False):
    """Returns (output, BassKernelResults)."""
    from concourse.bass_utils import run_bass_kernel_spmd
    from concourse.bass_interp import get_hw_module

    x = np.ascontiguousarray(np.asarray(x, dtype=np.float32))
    kb = np.ascontiguousarray(
        np.asarray(martx, dtype=np.float32).reshape(HW, KD)
    )
    a_val = float(np.asarray(aphal).reshape(-1)[0])
    g_val = float(np.asarray(gamma).reshape(-1)[0])

    nc = _get_program(a_val, g_val)
    shards = x.reshape(N_CORES, ROWS, HW)
    in_maps = [{"x": shards[i]} for i in range(N_CORES)]
    if not DEFAULT_CFG.get("kbT_only"):
        for m in in_maps:
            m["kb"] = kb
    if DEFAULT_CFG.get("kbT_only") or DEFAULT_CFG.get("host_kbT"):
        kbT = np.ascontiguousarray(kb.T)
        for m in in_maps:
            m["kbT"] = kbT

    old_m = nc.m
    nc.m = get_hw_module(nc.m)
    try:
        res = run_bass_kernel_spmd(
            nc, in_maps, core_ids=list(range(N_CORES)), trace=trace
        )
    finally:
        nc.m = old_m

    out = np.stack([res.results[i]["out"] for i in range(N_CORES)])
    out = out.reshape(B, C, 64, 64).astype(np.float32)
    return out, res


def kernel(x, martx, aphal, gamma):
    out, _ = run(x, martx, aphal, gamma, trace=False)
    return out

